# revision 1
# baseline (speedup 1.0000x reference)
"""GRU cell kernel for Trainium2, 8-core data-parallel.

Layout strategy: all activations are staged feature-major ([128, B]) in HBM by
the host, so the device kernel streams them directly as matmul moving operands
(contraction over the 128-feature partition dim) and applies per-partition
biases fused into the ScalarE activations.  Total HBM traffic is identical to
the batch-major layout; no on-chip transposes are needed.

Precision knobs (module CONFIG): gate intermediates (z, r, hc) and the h-side
matmul operands can run in bf16; x/h can additionally be staged bf16 in HBM to
cut DMA traffic. The final blend add and the output stay fp32.
"""

from contextlib import ExitStack

import numpy as np

B = 131072
H = 128
NCORES = 8
BC = B // NCORES  # 16384 batch rows per core
CHUNK = 512  # batch columns per tile (max fp32 matmul free dim / PSUM bank)
NCHUNK = BC // CHUNK

# Precision / tiling configuration used by both the program builder and the
# host-side staging in kernel().
CONFIG = {
    "inter_bf16": True,  # z, r, hc, rh, d, m tiles in bf16
    "h_bf16": True,  # stage h_prev as bf16 in HBM (halves h traffic)
    "x_bf16": True,  # stage x_t as bf16 in HBM (halves x traffic)
    "out_bf16": True,  # store h_t as bf16, upcast on host (halves out traffic)
    "free": 1024,  # ACT/DVE/DMA tile width (2 PSUM banks per gate tensor)
    "io_bufs": 6,
    "mid_bufs": 6,
}

_CACHE = {}
LAST_RESULTS = None


def _build_program(n_passes=1, mode="full", cfg=None):
    import concourse.bass as bass
    import concourse.tile as tile
    from concourse import bacc, mybir

    cfg = dict(CONFIG, **(cfg or {}))
    inter_bf16 = cfg["inter_bf16"]
    h_bf16 = cfg["h_bf16"]
    x_bf16 = cfg["x_bf16"]
    if h_bf16 or x_bf16:
        inter_bf16 = True

    f32 = mybir.dt.float32
    bf16 = mybir.dt.bfloat16
    x_dt = bf16 if x_bf16 else f32
    h_dt = bf16 if h_bf16 else f32
    mid_dt = bf16 if inter_bf16 else f32
    o_dt = bf16 if cfg.get("out_bf16") else f32

    Sig = mybir.ActivationFunctionType.Sigmoid
    Tanh = mybir.ActivationFunctionType.Tanh
    Mult = mybir.AluOpType.mult
    Sub = mybir.AluOpType.subtract
    Add = mybir.AluOpType.add

    nc = bacc.Bacc(
        "TRN2",
        target_bir_lowering=False,
        debug=False,
        enable_asserts=False,
        num_devices=NCORES,
    )

    xT = nc.dram_tensor("xT", [H, BC], x_dt, kind="ExternalInput").ap()
    hT = nc.dram_tensor("hT", [H, BC], h_dt, kind="ExternalInput").ap()
    # Wz, Uz, Wr, Ur, Wh, Uh stacked on the middle dim; natural [K=in, M=out]
    # layout is exactly the lhsT the tensor engine wants.
    W = nc.dram_tensor("W", [H, 6, H], f32, kind="ExternalInput").ap()
    bias = nc.dram_tensor("bias", [H, 3], f32, kind="ExternalInput").ap()
    oT = nc.dram_tensor("oT", [H, BC], o_dt, kind="ExternalOutput").ap()

    with tile.TileContext(nc) as tc:
        with ExitStack() as ctx:
            consts = ctx.enter_context(tc.tile_pool(name="consts", bufs=1))
            io = ctx.enter_context(tc.tile_pool(name="io", bufs=cfg["io_bufs"]))
            mid = ctx.enter_context(tc.tile_pool(name="mid", bufs=cfg["mid_bufs"]))
            psum = ctx.enter_context(tc.tile_pool(name="psum", bufs=2, space="PSUM"))

            w_s = consts.tile([H, 6, H], f32)
            nc.sync.dma_start(w_s[:], W)
            b_s = consts.tile([H, 3], f32)
            nc.sync.dma_start(b_s[:], bias)

            # bf16 copies of the stationary weights for any bf16-moving matmul
            # (a matmul may not mix fp32 with non-fp32 operands).
            if inter_bf16:
                w_b = consts.tile([H, 6, H], bf16)
                for i in range(6):
                    nc.vector.tensor_copy(w_b[:, i, :], w_s[:, i, :])

            def lhs(i, moving_dt):
                if moving_dt == f32:
                    return w_s[:, i, :]
                return w_b[:, i, :]

            Wz_i, Uz_i, Wr_i, Ur_i, Wh_i, Uh_i = range(6)
            bz, br, bh = (b_s[:, i : i + 1] for i in range(3))

            FREE = cfg.get("free", CHUNK)  # ACT/DVE/DMA tile width
            n_tiles = BC // FREE
            # Each matmul free dim stays <=512 (one PSUM bank); ACT/DVE/DMA
            # ops run on the full FREE width to amortize fixed overheads.
            mm_slices = [slice(s, s + CHUNK) for s in range(0, FREE, CHUNK)]
            # PSUM budget is 8 banks of 512 fp32: FREE=1024 -> pz/pr 1 buf,
            # ph 2 bufs (2+2+4 banks); FREE=512 -> 2 bufs each (6 banks).
            zr_bufs = 1 if FREE > CHUNK else 2
            carry = None

            def emit_tail(s):
                for ss in mm_slices:
                    nc.tensor.matmul(
                        s["ph"][:, ss], lhs(Uh_i, mid_dt), s["rh"][:, ss],
                        start=False, stop=True,
                    )
                hc = mid.tile([H, FREE], mid_dt, tag="hc")
                nc.scalar.activation(hc[:], s["ph"][:], Tanh, bias=bh)
                # h_t = h + z * (hc - h)
                d = mid.tile([H, FREE], mid_dt, tag="d")
                nc.vector.tensor_tensor(d[:], hc[:], s["htb"][:], Sub)
                m = mid.tile([H, FREE], mid_dt, tag="m")
                nc.vector.tensor_tensor(m[:], s["z"][:], d[:], Mult)
                o = mid.tile([H, FREE], o_dt, tag="o")
                nc.vector.tensor_tensor(o[:], s["ht"][:], m[:], Add)
                nc.sync.dma_start(oT[:, s["sl"]], o[:])

            for c in range(n_tiles * n_passes):
                c = c % n_tiles
                sl = bass.ts(c, FREE)
                xt = io.tile([H, FREE], x_dt, tag="xt")
                nc.sync.dma_start(xt[:], xT[:, sl])
                ht = io.tile([H, FREE], h_dt, tag="ht")
                nc.sync.dma_start(ht[:], hT[:, sl])

                if mode == "dma":
                    # DMA-only ablation: same loads, store alternates source
                    # so neither load is dead.
                    src = xt if (c % 2 == 0) else ht
                    o = mid.tile([H, FREE], o_dt, tag="o")
                    nc.vector.tensor_copy(o[:], src[:])
                    nc.sync.dma_start(oT[:, sl], o[:])
                    continue

                # bf16 copies of the moving operands so every matmul runs at
                # bf16 rate (fp32 matmuls are ~3x slower on the PE array).
                if inter_bf16 and not h_bf16:
                    htb = mid.tile([H, FREE], mid_dt, tag="htb")
                    nc.vector.tensor_copy(htb[:], ht[:])
                else:
                    htb = ht
                if inter_bf16 and not x_bf16:
                    xtb = mid.tile([H, FREE], mid_dt, tag="xtb")
                    nc.vector.tensor_copy(xtb[:], xt[:])
                else:
                    xtb = xt

                # z_pre.T = Wz.T x.T + Uz.T h.T  (K-split accumulation in PSUM)
                pz = psum.tile([H, FREE], f32, tag="pz", bufs=zr_bufs)
                pr = psum.tile([H, FREE], f32, tag="pr", bufs=zr_bufs)
                ph = psum.tile([H, FREE], f32, tag="ph", bufs=2)
                # Grouped by gate so each gate's full PSUM tensor finishes as
                # early as possible (its sigmoid can start while the next
                # gate's matmuls run).
                for ss in mm_slices:
                    nc.tensor.matmul(pz[:, ss], lhs(Wz_i, mid_dt), xtb[:, ss], start=True, stop=False)
                    nc.tensor.matmul(pz[:, ss], lhs(Uz_i, mid_dt), htb[:, ss], start=False, stop=True)
                for ss in mm_slices:
                    nc.tensor.matmul(pr[:, ss], lhs(Wr_i, mid_dt), xtb[:, ss], start=True, stop=False)
                    nc.tensor.matmul(pr[:, ss], lhs(Ur_i, mid_dt), htb[:, ss], start=False, stop=True)
                for ss in mm_slices:
                    nc.tensor.matmul(ph[:, ss], lhs(Wh_i, mid_dt), xtb[:, ss], start=True, stop=False)

                z = mid.tile([H, FREE], mid_dt, tag="z")
                nc.scalar.activation(z[:], pz[:], Sig, bias=bz)
                r = mid.tile([H, FREE], mid_dt, tag="r")
                nc.scalar.activation(r[:], pr[:], Sig, bias=br)

                rh = mid.tile([H, FREE], mid_dt, tag="rh")
                nc.vector.tensor_tensor(rh[:], r[:], htb[:], Mult)

                # Software-pipeline the Uh+tanh+blend stage by one tile: the
                # previous tile's Uh matmuls are emitted after this tile's
                # gate matmuls, so the in-order PE stream never stalls waiting
                # for the DVE rh of the tile it is currently computing.
                if carry is not None:
                    emit_tail(carry)
                carry = dict(ph=ph, rh=rh, z=z, ht=ht, htb=htb, sl=sl)
            if carry is not None:
                emit_tail(carry)

    nc.compile()
    return nc


def _get_program(n_passes=1, mode="full", cfg=None):
    key = (n_passes, mode, tuple(sorted((cfg or CONFIG).items())))
    if key not in _CACHE:
        _CACHE[key] = _build_program(n_passes, mode, cfg)
    return _CACHE[key]


def _np_dtypes():
    import ml_dtypes

    x_dt = ml_dtypes.bfloat16 if CONFIG["x_bf16"] else np.float32
    h_dt = ml_dtypes.bfloat16 if CONFIG["h_bf16"] else np.float32
    return x_dt, h_dt


def make_in_maps(x_t, h_prev, Wz, Uz, bz, Wr, Ur, br, Wh, Uh, bh):
    x_dt, h_dt = _np_dtypes()
    W = np.empty((H, 6, H), dtype=np.float32)
    for i, w in enumerate((Wz, Uz, Wr, Ur, Wh, Uh)):
        W[:, i, :] = np.asarray(w, dtype=np.float32)
    bias = np.empty((H, 3), dtype=np.float32)
    for i, b in enumerate((bz, br, bh)):
        bias[:, i] = np.asarray(b, dtype=np.float32)

    # Feature-major staging; the transpose happens on the host, outside the
    # device kernel. HBM bytes moved are unchanged (or reduced by bf16 knobs).
    xT = np.ascontiguousarray(np.asarray(x_t, dtype=np.float32).T.astype(x_dt))
    hT = np.ascontiguousarray(np.asarray(h_prev, dtype=np.float32).T.astype(h_dt))

    in_maps = []
    for c in range(NCORES):
        sl = slice(c * BC, (c + 1) * BC)
        in_maps.append(
            {
                "xT": np.ascontiguousarray(xT[:, sl]),
                "hT": np.ascontiguousarray(hT[:, sl]),
                "W": W,
                "bias": bias,
            }
        )
    return in_maps


def kernel(x_t, h_prev, Wz, Uz, bz, Wr, Ur, br, Wh, Uh, bh):
    global LAST_RESULTS
    from concourse import bass_utils

    in_maps = make_in_maps(x_t, h_prev, Wz, Uz, bz, Wr, Ur, br, Wh, Uh, bh)
    nc = _get_program()
    res = bass_utils.run_bass_kernel_spmd(nc, in_maps, core_ids=list(range(NCORES)))
    LAST_RESULTS = res

    oT = np.concatenate([r["oT"] for r in res.results], axis=1)  # [H, B]
    return np.ascontiguousarray(oT.T.astype(np.float32))

